# revision 13
# baseline (speedup 1.0000x reference)
"""Affine coupling layer (RealNVP-style) forward on 8 Trainium2 NeuronCores.

Pure data parallel: z is sharded along batch across 8 cores, the small MLP
weights are replicated.  Per core, activations are kept feature-major
([features, rows]) so the four Linear layers chain on the TensorEngine with
no inter-layer transposes; z tiles are transposed in/out with PE-mode
transpose.  Matmuls run in float32r (full PE rate at N>=256); the log_det
reduction runs in plain fp32 for precision.
"""

from contextlib import ExitStack

import numpy as np

import concourse.bass as bass
import concourse.bacc as bacc
import concourse.mybir as mybir
import concourse.tile as tile
from concourse.bass_utils import run_bass_kernel_spmd
from concourse.masks import make_identity

B, DIM, D_IN, HID = 65536, 128, 64, 512
D_OUT = DIM - D_IN
N_CORES = 8
ROWS = B // N_CORES          # rows per core
RT = 512                     # rows per tile (matmul free dim)
CLAMP = 2.0

F32 = mybir.dt.float32
F32R = mybir.dt.float32r
AT = mybir.ActivationFunctionType
OP = mybir.AluOpType

KC = HID // 128              # k-chunks for a 512-wide input
MC = HID // 128              # m-chunks for a 512-wide output


def build_nc(rows=ROWS, rt=RT):
    nrt = rows // rt
    nc = bacc.Bacc("TRN2", target_bir_lowering=False, debug=False,
                   num_devices=N_CORES)

    z = nc.dram_tensor("z", [rows, DIM], F32, kind="ExternalInput")
    w = {}
    for p in ("s", "t"):
        w[p + "W1T"] = nc.dram_tensor(p + "W1T", [128, HID], F32R, kind="ExternalInput")
        w[p + "W2T"] = nc.dram_tensor(p + "W2T", [HID, HID], F32R, kind="ExternalInput")
        w[p + "W3T"] = nc.dram_tensor(p + "W3T", [HID, HID], F32R, kind="ExternalInput")
        w[p + "W4T"] = nc.dram_tensor(p + "W4T", [HID, D_OUT], F32R, kind="ExternalInput")
        w[p + "b1"] = nc.dram_tensor(p + "b1", [HID], F32, kind="ExternalInput")
        w[p + "b2"] = nc.dram_tensor(p + "b2", [HID], F32, kind="ExternalInput")
        w[p + "b3"] = nc.dram_tensor(p + "b3", [HID], F32, kind="ExternalInput")
        w[p + "b4"] = nc.dram_tensor(p + "b4", [D_OUT], F32, kind="ExternalInput")
    z_out = nc.dram_tensor("z_out", [rows, DIM], F32, kind="ExternalOutput")
    log_det = nc.dram_tensor("log_det", [rows], F32, kind="ExternalOutput")

    with tile.TileContext(nc) as tc, ExitStack() as ctx:
        consts = ctx.enter_context(tc.tile_pool(name="consts", bufs=1))
        io = ctx.enter_context(tc.tile_pool(name="io", bufs=4))
        acts = ctx.enter_context(tc.tile_pool(name="acts", bufs=2))
        hpool = ctx.enter_context(tc.tile_pool(name="hpool", bufs=12))
        psum = ctx.enter_context(tc.tile_pool(name="psum", bufs=2, space="PSUM"))

        # --- constants ---
        ident = consts.tile([128, 128], F32, name="ident")
        make_identity(nc, ident)
        ones64_f = consts.tile([D_IN, 1], F32, name="ones64_f")
        nc.vector.memset(ones64_f, 1.0)
        ones64 = consts.tile([D_IN, 1], F32R, name="ones64")
        nc.vector.tensor_copy(ones64, ones64_f)

        W = {}
        for p in ("s", "t"):
            w1t = consts.tile([128, HID], F32R, name=f"{p}w1t")
            nc.sync.dma_start(out=w1t, in_=w[p + "W1T"][:, :])
            w2t = []
            w3t = []
            w4t = []
            for k in range(KC):
                t2 = consts.tile([128, HID], F32R, name=f"{p}w2t{k}")
                nc.sync.dma_start(out=t2, in_=w[p + "W2T"][128 * k : 128 * (k + 1), :])
                w2t.append(t2)
                t3 = consts.tile([128, HID], F32R, name=f"{p}w3t{k}")
                nc.sync.dma_start(out=t3, in_=w[p + "W3T"][128 * k : 128 * (k + 1), :])
                w3t.append(t3)
                t4 = consts.tile([128, D_OUT], F32R, name=f"{p}w4t{k}")
                nc.sync.dma_start(out=t4, in_=w[p + "W4T"][128 * k : 128 * (k + 1), :])
                w4t.append(t4)
            b1 = consts.tile([128, MC], F32, name=f"{p}b1t")
            nc.sync.dma_start(out=b1, in_=w[p + "b1"][:].rearrange("(m p) -> p m", p=128))
            b2 = consts.tile([128, MC], F32, name=f"{p}b2t")
            nc.sync.dma_start(out=b2, in_=w[p + "b2"][:].rearrange("(m p) -> p m", p=128))
            b3 = consts.tile([128, MC], F32, name=f"{p}b3t")
            nc.sync.dma_start(out=b3, in_=w[p + "b3"][:].rearrange("(m p) -> p m", p=128))
            b4 = consts.tile([D_OUT, 1], F32, name=f"{p}b4t")
            nc.sync.dma_start(out=b4, in_=w[p + "b4"][:].rearrange("(d o) -> d o", o=1))
            W[p] = dict(w1t=w1t, w2t=w2t, w3t=w3t, w4t=w4t, b1=b1, b2=b2, b3=b3, b4=b4)

        cpr = rt // 128  # 128-row chunks per row tile

        def hidden_layer(wk, b, h_in, relu_on_act, it, p, li):
            """h_out[m] = relu(sum_k wk[k][:,m].T @ h_in[k] + b[:,m]); k-outer."""
            nK = len(h_in)
            ps = [
                psum.tile([128, rt], F32, name=f"ps_{p}{li}_{it}_{m}", tag="hps", bufs=5)
                for m in range(MC)
            ]
            for k in range(nK):
                for m in range(MC):
                    nc.tensor.matmul(ps[m], wk[k][:, 128 * m : 128 * (m + 1)], h_in[k],
                                     start=(k == 0), stop=(k == nK - 1))
            h_out = []
            for m in range(MC):
                h = hpool.tile([128, rt], F32R, name=f"h_{p}{li}_{it}_{m}", tag="h")
                if m % 2 == 0:
                    nc.scalar.activation(h, ps[m], AT.Relu, bias=b[:, m : m + 1])
                else:
                    nc.vector.tensor_scalar(h, ps[m], b[:, m : m + 1], 0.0, OP.add, OP.max)
                h_out.append(h)
            return h_out

        def emit_dma_in(it):
            r0 = it * rt
            zrow = io.tile([128, cpr, DIM], F32, name=f"zrow_{it}", tag="zrow")
            nc.sync.dma_start(
                out=zrow, in_=z[r0 : r0 + rt, :].rearrange("(c p) d -> p c d", p=128)
            )
            return zrow

        def emit_A(it, zrow):
            """Input transposes + feature-major copies (PE + DVE, off critical path)."""
            tpz = psum.tile([128, rt], F32, name=f"tpz_{it}", tag="small", bufs=2)
            for c in range(cpr):
                nc.tensor.transpose(tpz[:, 128 * c : 128 * (c + 1)], zrow[:, c, :], ident)
            zT = acts.tile([128, rt], F32R, name=f"zT_{it}", tag="zT", bufs=3)
            nc.vector.tensor_copy(zT, tpz)
            tpz2 = psum.tile([D_OUT, rt], F32, name=f"tpz2_{it}", tag="small", bufs=2)
            for c in range(cpr):
                nc.tensor.transpose(tpz2[:, 128 * c : 128 * (c + 1)],
                                    zrow[:, c, D_IN:DIM], ident)
            z2T = acts.tile([D_OUT, rt], F32, name=f"z2T_{it}", tag="z2T", bufs=3)
            nc.vector.tensor_copy(z2T, tpz2)
            return dict(zrow=zrow, zT=zT, z2T=z2T)

        def emit_L1(it, zT):
            h1 = {}
            for p in ("s", "t"):
                ww = W[p]
                ps1 = [
                    psum.tile([128, rt], F32, name=f"ps1_{p}_{it}_{m}", tag="hps", bufs=5)
                    for m in range(MC)
                ]
                for m in range(MC):
                    nc.tensor.matmul(ps1[m], ww["w1t"][:, 128 * m : 128 * (m + 1)], zT)
                hs = []
                for m in range(MC):
                    h = hpool.tile([128, rt], F32R, name=f"h1_{p}_{it}_{m}", tag="h")
                    if m % 2 == 0:
                        nc.scalar.activation(h, ps1[m], AT.Relu, bias=ww["b1"][:, m : m + 1])
                    else:
                        nc.vector.tensor_scalar(h, ps1[m], ww["b1"][:, m : m + 1], 0.0,
                                                OP.add, OP.max)
                    hs.append(h)
                h1[p] = hs
            return h1

        def emit_L4(it, p, h3):
            ps4 = psum.tile([D_OUT, rt], F32, name=f"ps4_{p}_{it}", tag="small", bufs=2)
            for k in range(KC):
                nc.tensor.matmul(ps4, W[p]["w4t"][k], h3[k],
                                 start=(k == 0), stop=(k == KC - 1))
            return ps4

        def emit_heads(it, a, outs):
            """s/t heads + elementwise tail — ScalarE/VectorE only, no PE."""
            s_pre = acts.tile([D_OUT, rt], F32, name=f"s_pre_{it}", tag="s_pre")
            nc.scalar.activation(s_pre, outs["s"], AT.Identity, bias=W["s"]["b4"][:, 0:1])
            log_s = acts.tile([D_OUT, rt], F32R, name=f"log_s_{it}", tag="log_s")
            nc.vector.tensor_scalar(log_s, s_pre, CLAMP, -CLAMP, OP.min, OP.max)
            exp_s = acts.tile([D_OUT, rt], F32, name=f"exp_s_{it}", tag="exp_s")
            nc.scalar.activation(exp_s, log_s, AT.Exp)
            tvals = acts.tile([D_OUT, rt], F32, name=f"tvals_{it}", tag="tvals")
            nc.scalar.activation(tvals, outs["t"], AT.Identity, bias=W["t"]["b4"][:, 0:1])
            zmul = acts.tile([D_OUT, rt], F32, name=f"zmul_{it}", tag="zmul")
            nc.vector.tensor_mul(zmul, a["z2T"], exp_s)
            z2o = acts.tile([D_OUT, rt], F32, name=f"z2o_{it}", tag="z2o")
            nc.vector.tensor_add(z2o, zmul, tvals)
            return dict(log_s=log_s, z2o=z2o)

        def emit_C(it, a, hd):
            """log_det reduce + output transposes + stores (deps long since ready)."""
            r0 = it * rt
            ldp = psum.tile([1, rt], F32, name=f"ldp_{it}", tag="mis", bufs=1)
            nc.tensor.matmul(ldp, ones64, hd["log_s"])  # f32r column sum
            ld_sb = acts.tile([1, rt], F32, name=f"ld_sb_{it}", tag="ld")
            nc.vector.tensor_copy(ld_sb, ldp)
            nc.sync.dma_start(out=log_det[r0 : r0 + rt], in_=ld_sb)
            pso = psum.tile([128, cpr * D_OUT], F32, name=f"pso_{it}", tag="mis", bufs=1)
            for c in range(cpr):
                nc.tensor.transpose(
                    pso[:, D_OUT * c : D_OUT * (c + 1)],
                    hd["z2o"][:, 128 * c : 128 * (c + 1)],
                    ident[0:D_OUT, 0:D_OUT],
                )
            zoutt = io.tile([128, cpr, DIM], F32, name=f"zoutt_{it}", tag="zout")
            for c in range(cpr):
                nc.gpsimd.tensor_copy(zoutt[:, c, 0:D_IN], a["zrow"][:, c, 0:D_IN])
                nc.vector.tensor_copy(
                    zoutt[:, c, D_IN:DIM], pso[:, D_OUT * c : D_OUT * (c + 1)]
                )
            nc.sync.dma_start(
                out=z_out[r0 : r0 + rt, :].rearrange("(c p) d -> p c d", p=128),
                in_=zoutt,
            )

        # software pipeline: C(it-1) and A(it+1) are emitted inside tile it's
        # matmul stream so PE always has dense, dependency-ready work.
        zrow0 = emit_dma_in(0)
        a = emit_A(0, zrow0)
        a_next = None
        prevC = None
        for it in range(nrt):
            if it + 1 < nrt:
                zrow_next = emit_dma_in(it + 1)
            h1 = emit_L1(it, a["zT"])
            h2s = hidden_layer(W["s"]["w2t"], W["s"]["b2"], h1["s"], True, it, "s", 2)
            if prevC is not None:
                emit_C(*prevC)
                prevC = None
            h2t = hidden_layer(W["t"]["w2t"], W["t"]["b2"], h1["t"], True, it, "t", 2)
            h3s = hidden_layer(W["s"]["w3t"], W["s"]["b3"], h2s, True, it, "s", 3)
            h3t = hidden_layer(W["t"]["w3t"], W["t"]["b3"], h2t, True, it, "t", 3)
            if it + 1 < nrt:
                a_next = emit_A(it + 1, zrow_next)
            outs = {"s": emit_L4(it, "s", h3s), "t": emit_L4(it, "t", h3t)}
            hd = emit_heads(it, a, outs)
            prevC = (it, a, hd)
            a = a_next
        emit_C(*prevC)

    nc.compile()
    return nc


def _host_inputs(z, s_Win, s_bin, s_Whid, s_bhid, s_Wout, s_bout,
                 t_Win, t_bin, t_Whid, t_bhid, t_Wout, t_bout):
    f = lambda x: np.ascontiguousarray(np.asarray(x, dtype=np.float32))
    shared = {}
    for p, Win, bin_, Whid, bhid, Wout, bout in (
        ("s", s_Win, s_bin, s_Whid, s_bhid, s_Wout, s_bout),
        ("t", t_Win, t_bin, t_Whid, t_bhid, t_Wout, t_bout),
    ):
        w1t = np.zeros((128, HID), np.float32)
        w1t[:D_IN] = np.asarray(Win).T
        shared[p + "W1T"] = w1t
        shared[p + "W2T"] = f(np.asarray(Whid)[0].T)
        shared[p + "W3T"] = f(np.asarray(Whid)[1].T)
        shared[p + "W4T"] = f(np.asarray(Wout).T)
        shared[p + "b1"] = f(bin_)
        shared[p + "b2"] = f(np.asarray(bhid)[0])
        shared[p + "b3"] = f(np.asarray(bhid)[1])
        shared[p + "b4"] = f(bout)
    z = f(z)
    return z, shared


_NC_CACHE = {}


def _get_nc(rows=ROWS, rt=RT):
    key = (rows, rt)
    if key not in _NC_CACHE:
        _NC_CACHE[key] = build_nc(rows, rt)
    return _NC_CACHE[key]


def run_on_device(z, shared, trace=False, **kw):
    nc = _get_nc()
    in_maps = [
        {"z": z[i * ROWS : (i + 1) * ROWS], **shared} for i in range(N_CORES)
    ]
    res = run_bass_kernel_spmd(nc, in_maps, core_ids=list(range(N_CORES)),
                               trace=trace, **kw)
    z_out = np.concatenate([r["z_out"] for r in res.results], axis=0)
    log_det = np.concatenate([r["log_det"] for r in res.results], axis=0)
    return (z_out, log_det), res


def kernel(**inputs):
    z, shared = _host_inputs(**inputs)
    out, _ = run_on_device(z, shared, trace=False)
    return out


# revision 14
# speedup vs baseline: 1.1319x; 1.1319x over previous
"""Affine coupling layer (RealNVP-style) forward on 8 Trainium2 NeuronCores.

Pure data parallel: z is sharded along batch across 8 cores, the small MLP
weights are replicated.  Per core, activations are kept feature-major
([features, rows]) so the four Linear layers chain on the TensorEngine with
no inter-layer transposes; z tiles are transposed in/out with PE-mode
transpose.  Matmuls run in float32r (full PE rate at N>=256); the log_det
reduction runs in plain fp32 for precision.
"""

from contextlib import ExitStack

import numpy as np

import concourse.bass as bass
import concourse.bacc as bacc
import concourse.mybir as mybir
import concourse.tile as tile
from concourse.bass_utils import run_bass_kernel_spmd
from concourse.masks import make_identity

B, DIM, D_IN, HID = 65536, 128, 64, 512
D_OUT = DIM - D_IN
N_CORES = 8
ROWS = B // N_CORES          # rows per core
RT = 512                     # rows per tile (matmul free dim)
CLAMP = 2.0

F32 = mybir.dt.float32
F32R = mybir.dt.float32r
AT = mybir.ActivationFunctionType
OP = mybir.AluOpType

KC = HID // 128              # k-chunks for a 512-wide input
MC = HID // 128              # m-chunks for a 512-wide output


def build_nc(rows=ROWS, rt=RT):
    nrt = rows // rt
    nc = bacc.Bacc("TRN2", target_bir_lowering=False, debug=False,
                   num_devices=N_CORES)

    z = nc.dram_tensor("z", [rows, DIM], F32, kind="ExternalInput")
    w = {}
    for p in ("s", "t"):
        w[p + "W1T"] = nc.dram_tensor(p + "W1T", [128, HID], F32R, kind="ExternalInput")
        w[p + "W2T"] = nc.dram_tensor(p + "W2T", [HID, HID], F32R, kind="ExternalInput")
        w[p + "W3T"] = nc.dram_tensor(p + "W3T", [HID, HID], F32R, kind="ExternalInput")
        w[p + "W4T"] = nc.dram_tensor(p + "W4T", [HID, D_OUT], F32R, kind="ExternalInput")
        w[p + "b1"] = nc.dram_tensor(p + "b1", [HID], F32, kind="ExternalInput")
        w[p + "b2"] = nc.dram_tensor(p + "b2", [HID], F32, kind="ExternalInput")
        w[p + "b3"] = nc.dram_tensor(p + "b3", [HID], F32, kind="ExternalInput")
        w[p + "b4"] = nc.dram_tensor(p + "b4", [D_OUT], F32, kind="ExternalInput")
    z_out = nc.dram_tensor("z_out", [rows, DIM], F32, kind="ExternalOutput")
    log_det = nc.dram_tensor("log_det", [rows], F32, kind="ExternalOutput")

    with tile.TileContext(nc) as tc, ExitStack() as ctx:
        consts = ctx.enter_context(tc.tile_pool(name="consts", bufs=1))
        io = ctx.enter_context(tc.tile_pool(name="io", bufs=4))
        acts = ctx.enter_context(tc.tile_pool(name="acts", bufs=2))
        hpool = ctx.enter_context(tc.tile_pool(name="hpool", bufs=12))
        psum = ctx.enter_context(tc.tile_pool(name="psum", bufs=2, space="PSUM"))

        # --- constants ---
        ident = consts.tile([128, 128], F32, name="ident")
        make_identity(nc, ident)
        ones64_f = consts.tile([D_IN, 1], F32, name="ones64_f")
        nc.vector.memset(ones64_f, 1.0)
        ones64 = consts.tile([D_IN, 1], F32R, name="ones64")
        nc.vector.tensor_copy(ones64, ones64_f)

        W = {}
        for p in ("s", "t"):
            w1t = consts.tile([128, HID], F32R, name=f"{p}w1t")
            nc.sync.dma_start(out=w1t, in_=w[p + "W1T"][:, :])
            w2t = []
            w3t = []
            w4t = []
            for k in range(KC):
                t2 = consts.tile([128, HID], F32R, name=f"{p}w2t{k}")
                nc.sync.dma_start(out=t2, in_=w[p + "W2T"][128 * k : 128 * (k + 1), :])
                w2t.append(t2)
                t3 = consts.tile([128, HID], F32R, name=f"{p}w3t{k}")
                nc.sync.dma_start(out=t3, in_=w[p + "W3T"][128 * k : 128 * (k + 1), :])
                w3t.append(t3)
                t4 = consts.tile([128, D_OUT], F32R, name=f"{p}w4t{k}")
                nc.sync.dma_start(out=t4, in_=w[p + "W4T"][128 * k : 128 * (k + 1), :])
                w4t.append(t4)
            b1 = consts.tile([128, MC], F32, name=f"{p}b1t")
            nc.sync.dma_start(out=b1, in_=w[p + "b1"][:].rearrange("(m p) -> p m", p=128))
            b2 = consts.tile([128, MC], F32, name=f"{p}b2t")
            nc.sync.dma_start(out=b2, in_=w[p + "b2"][:].rearrange("(m p) -> p m", p=128))
            b3 = consts.tile([128, MC], F32, name=f"{p}b3t")
            nc.sync.dma_start(out=b3, in_=w[p + "b3"][:].rearrange("(m p) -> p m", p=128))
            b4 = consts.tile([D_OUT, 1], F32, name=f"{p}b4t")
            nc.sync.dma_start(out=b4, in_=w[p + "b4"][:].rearrange("(d o) -> d o", o=1))
            W[p] = dict(w1t=w1t, w2t=w2t, w3t=w3t, w4t=w4t, b1=b1, b2=b2, b3=b3, b4=b4)

        cpr = rt // 128  # 128-row chunks per row tile

        half = rt // 2

        def relu_split(h, ps, b_ap):
            """PSUM->SBUF bias+relu split across ScalarE/VectorE for latency."""
            nc.scalar.activation(h[:, 0:half], ps[:, 0:half], AT.Relu, bias=b_ap)
            nc.vector.tensor_scalar(h[:, half:rt], ps[:, half:rt], b_ap, 0.0,
                                    OP.add, OP.max)

        def hidden_layer(wk, b, h_in, relu_on_act, it, p, li):
            """h_out[m] = relu(sum_k wk[k][:,m].T @ h_in[k] + b[:,m]).

            k-outer inside m-pairs: only 2 PSUM banks live per pair, so the
            next group never waits on relu drains."""
            nK = len(h_in)
            h_out = [None] * MC
            for mp in range(0, MC, 2):
                pair = (mp, mp + 1)
                ps = [
                    psum.tile([128, rt], F32, name=f"ps_{p}{li}_{it}_{m}",
                              tag="hps", bufs=5)
                    for m in pair
                ]
                for k in range(nK):
                    for j, m in enumerate(pair):
                        nc.tensor.matmul(ps[j], wk[k][:, 128 * m : 128 * (m + 1)],
                                         h_in[k], start=(k == 0), stop=(k == nK - 1))
                for j, m in enumerate(pair):
                    h = hpool.tile([128, rt], F32R, name=f"h_{p}{li}_{it}_{m}", tag="h")
                    relu_split(h, ps[j], b[:, m : m + 1])
                    h_out[m] = h
            return h_out

        def emit_dma_in(it):
            r0 = it * rt
            zrow = io.tile([128, cpr, DIM], F32, name=f"zrow_{it}", tag="zrow")
            nc.sync.dma_start(
                out=zrow, in_=z[r0 : r0 + rt, :].rearrange("(c p) d -> p c d", p=128)
            )
            return zrow

        def emit_A(it, zrow):
            """Input transposes + feature-major copies (PE + DVE, off critical path)."""
            tpz = psum.tile([128, rt], F32, name=f"tpz_{it}", tag="small", bufs=2)
            for c in range(cpr):
                nc.tensor.transpose(tpz[:, 128 * c : 128 * (c + 1)], zrow[:, c, :], ident)
            zT = acts.tile([128, rt], F32R, name=f"zT_{it}", tag="zT", bufs=3)
            nc.vector.tensor_copy(zT, tpz)
            tpz2 = psum.tile([D_OUT, rt], F32, name=f"tpz2_{it}", tag="small", bufs=2)
            for c in range(cpr):
                nc.tensor.transpose(tpz2[:, 128 * c : 128 * (c + 1)],
                                    zrow[:, c, D_IN:DIM], ident)
            z2T = acts.tile([D_OUT, rt], F32, name=f"z2T_{it}", tag="z2T", bufs=3)
            nc.vector.tensor_copy(z2T, tpz2)
            return dict(zrow=zrow, zT=zT, z2T=z2T)

        def emit_L1(it, zT):
            h1 = {}
            for p in ("s", "t"):
                ww = W[p]
                hs = [None] * MC
                for mp in range(0, MC, 2):
                    pair = (mp, mp + 1)
                    ps1 = [
                        psum.tile([128, rt], F32, name=f"ps1_{p}_{it}_{m}",
                                  tag="hps", bufs=5)
                        for m in pair
                    ]
                    for j, m in enumerate(pair):
                        nc.tensor.matmul(ps1[j], ww["w1t"][:, 128 * m : 128 * (m + 1)], zT)
                    for j, m in enumerate(pair):
                        h = hpool.tile([128, rt], F32R, name=f"h1_{p}_{it}_{m}", tag="h")
                        relu_split(h, ps1[j], ww["b1"][:, m : m + 1])
                        hs[m] = h
                h1[p] = hs
            return h1

        def emit_L4(it, p, h3):
            ps4 = psum.tile([D_OUT, rt], F32, name=f"ps4_{p}_{it}", tag="small", bufs=2)
            for k in range(KC):
                nc.tensor.matmul(ps4, W[p]["w4t"][k], h3[k],
                                 start=(k == 0), stop=(k == KC - 1))
            return ps4

        def emit_heads(it, a, outs):
            """s/t heads + elementwise tail — ScalarE/VectorE only, no PE."""
            s_pre = acts.tile([D_OUT, rt], F32, name=f"s_pre_{it}", tag="s_pre")
            nc.scalar.activation(s_pre, outs["s"], AT.Identity, bias=W["s"]["b4"][:, 0:1])
            log_s = acts.tile([D_OUT, rt], F32R, name=f"log_s_{it}", tag="log_s")
            nc.vector.tensor_scalar(log_s, s_pre, CLAMP, -CLAMP, OP.min, OP.max)
            exp_s = acts.tile([D_OUT, rt], F32, name=f"exp_s_{it}", tag="exp_s")
            nc.scalar.activation(exp_s, log_s, AT.Exp)
            tvals = acts.tile([D_OUT, rt], F32, name=f"tvals_{it}", tag="tvals")
            nc.scalar.activation(tvals, outs["t"], AT.Identity, bias=W["t"]["b4"][:, 0:1])
            zmul = acts.tile([D_OUT, rt], F32, name=f"zmul_{it}", tag="zmul")
            nc.vector.tensor_mul(zmul, a["z2T"], exp_s)
            z2o = acts.tile([D_OUT, rt], F32, name=f"z2o_{it}", tag="z2o")
            nc.vector.tensor_add(z2o, zmul, tvals)
            return dict(log_s=log_s, z2o=z2o)

        def emit_C(it, a, hd):
            """log_det reduce + output transposes + stores (deps long since ready)."""
            r0 = it * rt
            ldp = psum.tile([1, rt], F32, name=f"ldp_{it}", tag="mis", bufs=1)
            nc.tensor.matmul(ldp, ones64, hd["log_s"])  # f32r column sum
            ld_sb = acts.tile([1, rt], F32, name=f"ld_sb_{it}", tag="ld")
            nc.vector.tensor_copy(ld_sb, ldp)
            nc.sync.dma_start(out=log_det[r0 : r0 + rt], in_=ld_sb)
            pso = psum.tile([128, cpr * D_OUT], F32, name=f"pso_{it}", tag="mis", bufs=1)
            for c in range(cpr):
                nc.tensor.transpose(
                    pso[:, D_OUT * c : D_OUT * (c + 1)],
                    hd["z2o"][:, 128 * c : 128 * (c + 1)],
                    ident[0:D_OUT, 0:D_OUT],
                )
            zoutt = io.tile([128, cpr, DIM], F32, name=f"zoutt_{it}", tag="zout")
            for c in range(cpr):
                nc.gpsimd.tensor_copy(zoutt[:, c, 0:D_IN], a["zrow"][:, c, 0:D_IN])
                nc.vector.tensor_copy(
                    zoutt[:, c, D_IN:DIM], pso[:, D_OUT * c : D_OUT * (c + 1)]
                )
            nc.sync.dma_start(
                out=z_out[r0 : r0 + rt, :].rearrange("(c p) d -> p c d", p=128),
                in_=zoutt,
            )

        # software pipeline: C(it-1) and A(it+1) are emitted inside tile it's
        # matmul stream so PE always has dense, dependency-ready work.
        zrow0 = emit_dma_in(0)
        a = emit_A(0, zrow0)
        a_next = None
        prevC = None
        for it in range(nrt):
            if it + 1 < nrt:
                zrow_next = emit_dma_in(it + 1)
            h1 = emit_L1(it, a["zT"])
            h2s = hidden_layer(W["s"]["w2t"], W["s"]["b2"], h1["s"], True, it, "s", 2)
            if prevC is not None:
                emit_C(*prevC)
                prevC = None
            h2t = hidden_layer(W["t"]["w2t"], W["t"]["b2"], h1["t"], True, it, "t", 2)
            h3s = hidden_layer(W["s"]["w3t"], W["s"]["b3"], h2s, True, it, "s", 3)
            h3t = hidden_layer(W["t"]["w3t"], W["t"]["b3"], h2t, True, it, "t", 3)
            if it + 1 < nrt:
                a_next = emit_A(it + 1, zrow_next)
            outs = {"s": emit_L4(it, "s", h3s), "t": emit_L4(it, "t", h3t)}
            hd = emit_heads(it, a, outs)
            prevC = (it, a, hd)
            a = a_next
        emit_C(*prevC)

    nc.compile()
    return nc


def _host_inputs(z, s_Win, s_bin, s_Whid, s_bhid, s_Wout, s_bout,
                 t_Win, t_bin, t_Whid, t_bhid, t_Wout, t_bout):
    f = lambda x: np.ascontiguousarray(np.asarray(x, dtype=np.float32))
    shared = {}
    for p, Win, bin_, Whid, bhid, Wout, bout in (
        ("s", s_Win, s_bin, s_Whid, s_bhid, s_Wout, s_bout),
        ("t", t_Win, t_bin, t_Whid, t_bhid, t_Wout, t_bout),
    ):
        w1t = np.zeros((128, HID), np.float32)
        w1t[:D_IN] = np.asarray(Win).T
        shared[p + "W1T"] = w1t
        shared[p + "W2T"] = f(np.asarray(Whid)[0].T)
        shared[p + "W3T"] = f(np.asarray(Whid)[1].T)
        shared[p + "W4T"] = f(np.asarray(Wout).T)
        shared[p + "b1"] = f(bin_)
        shared[p + "b2"] = f(np.asarray(bhid)[0])
        shared[p + "b3"] = f(np.asarray(bhid)[1])
        shared[p + "b4"] = f(bout)
    z = f(z)
    return z, shared


_NC_CACHE = {}


def _get_nc(rows=ROWS, rt=RT):
    key = (rows, rt)
    if key not in _NC_CACHE:
        _NC_CACHE[key] = build_nc(rows, rt)
    return _NC_CACHE[key]


def run_on_device(z, shared, trace=False, **kw):
    nc = _get_nc()
    in_maps = [
        {"z": z[i * ROWS : (i + 1) * ROWS], **shared} for i in range(N_CORES)
    ]
    res = run_bass_kernel_spmd(nc, in_maps, core_ids=list(range(N_CORES)),
                               trace=trace, **kw)
    z_out = np.concatenate([r["z_out"] for r in res.results], axis=0)
    log_det = np.concatenate([r["log_det"] for r in res.results], axis=0)
    return (z_out, log_det), res


def kernel(**inputs):
    z, shared = _host_inputs(**inputs)
    out, _ = run_on_device(z, shared, trace=False)
    return out


# revision 15
# speedup vs baseline: 1.1347x; 1.0025x over previous
"""Affine coupling layer (RealNVP-style) forward on 8 Trainium2 NeuronCores.

Pure data parallel: z is sharded along batch across 8 cores, the small MLP
weights are replicated.  Per core, activations are kept feature-major
([features, rows]) so the four Linear layers chain on the TensorEngine with
no inter-layer transposes; z tiles are transposed in/out with PE-mode
transpose.  Matmuls run in float32r (full PE rate at N>=256); the log_det
reduction runs in plain fp32 for precision.
"""

from contextlib import ExitStack

import numpy as np

import concourse.bass as bass
import concourse.bacc as bacc
import concourse.mybir as mybir
import concourse.tile as tile
from concourse.bass_utils import run_bass_kernel_spmd
from concourse.masks import make_identity

B, DIM, D_IN, HID = 65536, 128, 64, 512
D_OUT = DIM - D_IN
N_CORES = 8
ROWS = B // N_CORES          # rows per core
RT = 512                     # rows per tile (matmul free dim)
CLAMP = 2.0

F32 = mybir.dt.float32
F32R = mybir.dt.float32r
AT = mybir.ActivationFunctionType
OP = mybir.AluOpType

KC = HID // 128              # k-chunks for a 512-wide input
MC = HID // 128              # m-chunks for a 512-wide output


def build_nc(rows=ROWS, rt=RT):
    nrt = rows // rt
    nc = bacc.Bacc("TRN2", target_bir_lowering=False, debug=False,
                   num_devices=N_CORES)

    z = nc.dram_tensor("z", [rows, DIM], F32, kind="ExternalInput")
    w = {}
    for p in ("s", "t"):
        w[p + "W1T"] = nc.dram_tensor(p + "W1T", [128, HID], F32R, kind="ExternalInput")
        w[p + "W2T"] = nc.dram_tensor(p + "W2T", [HID, HID], F32R, kind="ExternalInput")
        w[p + "W3T"] = nc.dram_tensor(p + "W3T", [HID, HID], F32R, kind="ExternalInput")
        w[p + "W4T"] = nc.dram_tensor(p + "W4T", [HID, D_OUT], F32R, kind="ExternalInput")
        w[p + "b1"] = nc.dram_tensor(p + "b1", [HID], F32, kind="ExternalInput")
        w[p + "b2"] = nc.dram_tensor(p + "b2", [HID], F32, kind="ExternalInput")
        w[p + "b3"] = nc.dram_tensor(p + "b3", [HID], F32, kind="ExternalInput")
        w[p + "b4"] = nc.dram_tensor(p + "b4", [D_OUT], F32, kind="ExternalInput")
    z_out = nc.dram_tensor("z_out", [rows, DIM], F32, kind="ExternalOutput")
    log_det = nc.dram_tensor("log_det", [rows], F32, kind="ExternalOutput")

    with tile.TileContext(nc) as tc, ExitStack() as ctx:
        consts = ctx.enter_context(tc.tile_pool(name="consts", bufs=1))
        io = ctx.enter_context(tc.tile_pool(name="io", bufs=4))
        acts = ctx.enter_context(tc.tile_pool(name="acts", bufs=2))
        hpool = ctx.enter_context(tc.tile_pool(name="hpool", bufs=12))
        psum = ctx.enter_context(tc.tile_pool(name="psum", bufs=2, space="PSUM"))

        # --- constants ---
        ident = consts.tile([128, 128], F32, name="ident")
        make_identity(nc, ident)
        ones64_f = consts.tile([D_IN, 1], F32, name="ones64_f")
        nc.vector.memset(ones64_f, 1.0)
        ones64 = consts.tile([D_IN, 1], F32R, name="ones64")
        nc.vector.tensor_copy(ones64, ones64_f)

        W = {}
        for p in ("s", "t"):
            w1t = consts.tile([128, HID], F32R, name=f"{p}w1t")
            nc.sync.dma_start(out=w1t, in_=w[p + "W1T"][:, :])
            w2t = []
            w3t = []
            w4t = []
            for k in range(KC):
                t2 = consts.tile([128, HID], F32R, name=f"{p}w2t{k}")
                nc.sync.dma_start(out=t2, in_=w[p + "W2T"][128 * k : 128 * (k + 1), :])
                w2t.append(t2)
                t3 = consts.tile([128, HID], F32R, name=f"{p}w3t{k}")
                nc.sync.dma_start(out=t3, in_=w[p + "W3T"][128 * k : 128 * (k + 1), :])
                w3t.append(t3)
                t4 = consts.tile([128, D_OUT], F32R, name=f"{p}w4t{k}")
                nc.sync.dma_start(out=t4, in_=w[p + "W4T"][128 * k : 128 * (k + 1), :])
                w4t.append(t4)
            b1 = consts.tile([128, MC], F32, name=f"{p}b1t")
            nc.sync.dma_start(out=b1, in_=w[p + "b1"][:].rearrange("(m p) -> p m", p=128))
            b2 = consts.tile([128, MC], F32, name=f"{p}b2t")
            nc.sync.dma_start(out=b2, in_=w[p + "b2"][:].rearrange("(m p) -> p m", p=128))
            b3 = consts.tile([128, MC], F32, name=f"{p}b3t")
            nc.sync.dma_start(out=b3, in_=w[p + "b3"][:].rearrange("(m p) -> p m", p=128))
            b4 = consts.tile([D_OUT, 1], F32, name=f"{p}b4t")
            nc.sync.dma_start(out=b4, in_=w[p + "b4"][:].rearrange("(d o) -> d o", o=1))
            W[p] = dict(w1t=w1t, w2t=w2t, w3t=w3t, w4t=w4t, b1=b1, b2=b2, b3=b3, b4=b4)

        cpr = rt // 128  # 128-row chunks per row tile

        half = rt // 2

        def relu_split(h, ps, b_ap):
            """PSUM->SBUF bias+relu split across ScalarE/VectorE for latency."""
            nc.scalar.activation(h[:, 0:half], ps[:, 0:half], AT.Relu, bias=b_ap)
            nc.vector.tensor_scalar(h[:, half:rt], ps[:, half:rt], b_ap, 0.0,
                                    OP.add, OP.max)

        def hidden_layer(wk, b, h_in, relu_on_act, it, p, li):
            """h_out[m] = relu(sum_k wk[k][:,m].T @ h_in[k] + b[:,m]).

            k-outer inside m-pairs: only 2 PSUM banks live per pair, so the
            next group never waits on relu drains."""
            nK = len(h_in)
            h_out = [None] * MC
            for mp in range(0, MC, 2):
                pair = (mp, mp + 1)
                ps = [
                    psum.tile([128, rt], F32, name=f"ps_{p}{li}_{it}_{m}",
                              tag="hps", bufs=5)
                    for m in pair
                ]
                for k in range(nK):
                    for j, m in enumerate(pair):
                        nc.tensor.matmul(ps[j], wk[k][:, 128 * m : 128 * (m + 1)],
                                         h_in[k], start=(k == 0), stop=(k == nK - 1))
                for j, m in enumerate(pair):
                    h = hpool.tile([128, rt], F32R, name=f"h_{p}{li}_{it}_{m}", tag="h")
                    relu_split(h, ps[j], b[:, m : m + 1])
                    h_out[m] = h
            return h_out

        def emit_dma_in(it):
            r0 = it * rt
            zrow = io.tile([128, cpr, DIM], F32, name=f"zrow_{it}", tag="zrow")
            nc.sync.dma_start(
                out=zrow, in_=z[r0 : r0 + rt, :].rearrange("(c p) d -> p c d", p=128)
            )
            return zrow

        def emit_A(it, zrow):
            """Input transposes + feature-major copies (PE + DVE, off critical path)."""
            tpz = psum.tile([128, rt], F32, name=f"tpz_{it}", tag="small", bufs=2)
            for c in range(cpr):
                nc.tensor.transpose(tpz[:, 128 * c : 128 * (c + 1)], zrow[:, c, :], ident)
            zT = acts.tile([128, rt], F32R, name=f"zT_{it}", tag="zT", bufs=3)
            nc.vector.tensor_copy(zT, tpz)
            tpz2 = psum.tile([D_OUT, rt], F32, name=f"tpz2_{it}", tag="small", bufs=2)
            for c in range(cpr):
                nc.tensor.transpose(tpz2[:, 128 * c : 128 * (c + 1)],
                                    zrow[:, c, D_IN:DIM], ident)
            z2T = acts.tile([D_OUT, rt], F32, name=f"z2T_{it}", tag="z2T", bufs=3)
            nc.vector.tensor_copy(z2T, tpz2)
            return dict(zrow=zrow, zT=zT, z2T=z2T)

        def emit_L1(it, zT):
            h1 = {}
            for p in ("s", "t"):
                ww = W[p]
                hs = [None] * MC
                for mp in range(0, MC, 2):
                    pair = (mp, mp + 1)
                    ps1 = [
                        psum.tile([128, rt], F32, name=f"ps1_{p}_{it}_{m}",
                                  tag="hps", bufs=5)
                        for m in pair
                    ]
                    for j, m in enumerate(pair):
                        nc.tensor.matmul(ps1[j], ww["w1t"][:, 128 * m : 128 * (m + 1)], zT)
                    for j, m in enumerate(pair):
                        h = hpool.tile([128, rt], F32R, name=f"h1_{p}_{it}_{m}", tag="h")
                        relu_split(h, ps1[j], ww["b1"][:, m : m + 1])
                        hs[m] = h
                h1[p] = hs
            return h1

        def emit_L4(it, p, h3):
            ps4 = psum.tile([D_OUT, rt], F32, name=f"ps4_{p}_{it}", tag="small", bufs=2)
            for k in range(KC):
                nc.tensor.matmul(ps4, W[p]["w4t"][k], h3[k],
                                 start=(k == 0), stop=(k == KC - 1))
            return ps4

        def emit_heads(it, a, outs):
            """s/t heads + elementwise tail — ScalarE/VectorE only, no PE."""
            s_pre = acts.tile([D_OUT, rt], F32, name=f"s_pre_{it}", tag="s_pre")
            nc.scalar.activation(s_pre, outs["s"], AT.Identity, bias=W["s"]["b4"][:, 0:1])
            log_s = acts.tile([D_OUT, rt], F32R, name=f"log_s_{it}", tag="log_s")
            nc.vector.tensor_scalar(log_s, s_pre, CLAMP, -CLAMP, OP.min, OP.max)
            exp_s = acts.tile([D_OUT, rt], F32, name=f"exp_s_{it}", tag="exp_s")
            nc.scalar.activation(exp_s, log_s, AT.Exp)
            tvals = acts.tile([D_OUT, rt], F32, name=f"tvals_{it}", tag="tvals")
            nc.scalar.activation(tvals, outs["t"], AT.Identity, bias=W["t"]["b4"][:, 0:1])
            zmul = acts.tile([D_OUT, rt], F32, name=f"zmul_{it}", tag="zmul")
            nc.vector.tensor_mul(zmul, a["z2T"], exp_s)
            z2o = acts.tile([D_OUT, rt], F32, name=f"z2o_{it}", tag="z2o")
            nc.vector.tensor_add(z2o, zmul, tvals)
            return dict(log_s=log_s, z2o=z2o)

        def emit_C(it, a, hd):
            """log_det reduce + output transposes + stores (deps long since ready)."""
            r0 = it * rt
            ldp = psum.tile([1, rt], F32, name=f"ldp_{it}", tag="mis", bufs=1)
            nc.tensor.matmul(ldp, ones64, hd["log_s"])  # f32r column sum
            ld_sb = acts.tile([1, rt], F32, name=f"ld_sb_{it}", tag="ld")
            nc.vector.tensor_copy(ld_sb, ldp)
            nc.sync.dma_start(out=log_det[r0 : r0 + rt], in_=ld_sb)
            pso = psum.tile([128, cpr * D_OUT], F32, name=f"pso_{it}", tag="mis", bufs=1)
            for c in range(cpr):
                nc.tensor.transpose(
                    pso[:, D_OUT * c : D_OUT * (c + 1)],
                    hd["z2o"][:, 128 * c : 128 * (c + 1)],
                    ident[0:D_OUT, 0:D_OUT],
                )
            zoutt = io.tile([128, cpr, DIM], F32, name=f"zoutt_{it}", tag="zout")
            for c in range(cpr):
                nc.gpsimd.tensor_copy(zoutt[:, c, 0:D_IN], a["zrow"][:, c, 0:D_IN])
                nc.vector.tensor_copy(
                    zoutt[:, c, D_IN:DIM], pso[:, D_OUT * c : D_OUT * (c + 1)]
                )
            nc.sync.dma_start(
                out=z_out[r0 : r0 + rt, :].rearrange("(c p) d -> p c d", p=128),
                in_=zoutt,
            )

        # software pipeline: C(it-1) and A(it+1) are emitted inside tile it's
        # matmul stream so PE always has dense, dependency-ready work.
        zrow0 = emit_dma_in(0)
        a = emit_A(0, zrow0)
        a_next = None
        prevC = None
        for it in range(nrt):
            if it + 1 < nrt:
                zrow_next = emit_dma_in(it + 1)
            h1 = emit_L1(it, a["zT"])
            h2s = hidden_layer(W["s"]["w2t"], W["s"]["b2"], h1["s"], True, it, "s", 2)
            h2t = hidden_layer(W["t"]["w2t"], W["t"]["b2"], h1["t"], True, it, "t", 2)
            if prevC is not None:
                emit_C(*prevC)
                prevC = None
            h3s = hidden_layer(W["s"]["w3t"], W["s"]["b3"], h2s, True, it, "s", 3)
            h3t = hidden_layer(W["t"]["w3t"], W["t"]["b3"], h2t, True, it, "t", 3)
            if it + 1 < nrt:
                a_next = emit_A(it + 1, zrow_next)
            outs = {"s": emit_L4(it, "s", h3s), "t": emit_L4(it, "t", h3t)}
            hd = emit_heads(it, a, outs)
            prevC = (it, a, hd)
            a = a_next
        emit_C(*prevC)

    nc.compile()
    return nc


def _host_inputs(z, s_Win, s_bin, s_Whid, s_bhid, s_Wout, s_bout,
                 t_Win, t_bin, t_Whid, t_bhid, t_Wout, t_bout):
    f = lambda x: np.ascontiguousarray(np.asarray(x, dtype=np.float32))
    shared = {}
    for p, Win, bin_, Whid, bhid, Wout, bout in (
        ("s", s_Win, s_bin, s_Whid, s_bhid, s_Wout, s_bout),
        ("t", t_Win, t_bin, t_Whid, t_bhid, t_Wout, t_bout),
    ):
        w1t = np.zeros((128, HID), np.float32)
        w1t[:D_IN] = np.asarray(Win).T
        shared[p + "W1T"] = w1t
        shared[p + "W2T"] = f(np.asarray(Whid)[0].T)
        shared[p + "W3T"] = f(np.asarray(Whid)[1].T)
        shared[p + "W4T"] = f(np.asarray(Wout).T)
        shared[p + "b1"] = f(bin_)
        shared[p + "b2"] = f(np.asarray(bhid)[0])
        shared[p + "b3"] = f(np.asarray(bhid)[1])
        shared[p + "b4"] = f(bout)
    z = f(z)
    return z, shared


_NC_CACHE = {}


def _get_nc(rows=ROWS, rt=RT):
    key = (rows, rt)
    if key not in _NC_CACHE:
        _NC_CACHE[key] = build_nc(rows, rt)
    return _NC_CACHE[key]


def run_on_device(z, shared, trace=False, **kw):
    nc = _get_nc()
    in_maps = [
        {"z": z[i * ROWS : (i + 1) * ROWS], **shared} for i in range(N_CORES)
    ]
    res = run_bass_kernel_spmd(nc, in_maps, core_ids=list(range(N_CORES)),
                               trace=trace, **kw)
    z_out = np.concatenate([r["z_out"] for r in res.results], axis=0)
    log_det = np.concatenate([r["log_det"] for r in res.results], axis=0)
    return (z_out, log_det), res


def kernel(**inputs):
    z, shared = _host_inputs(**inputs)
    out, _ = run_on_device(z, shared, trace=False)
    return out


# revision 16
# speedup vs baseline: 1.1584x; 1.0209x over previous
"""Affine coupling layer (RealNVP-style) forward on 8 Trainium2 NeuronCores.

Pure data parallel: z is sharded along batch across 8 cores, the small MLP
weights are replicated.  Per core, activations are kept feature-major
([features, rows]) so the four Linear layers chain on the TensorEngine with
no inter-layer transposes; z tiles are transposed in/out with PE-mode
transpose.  Matmuls run in float32r (full PE rate at N>=256); the log_det
reduction runs in plain fp32 for precision.
"""

from contextlib import ExitStack

import numpy as np

import concourse.bass as bass
import concourse.bacc as bacc
import concourse.mybir as mybir
import concourse.tile as tile
from concourse.bass_utils import run_bass_kernel_spmd
from concourse.masks import make_identity

B, DIM, D_IN, HID = 65536, 128, 64, 512
D_OUT = DIM - D_IN
N_CORES = 8
ROWS = B // N_CORES          # rows per core
RT = 512                     # rows per tile (matmul free dim)
CLAMP = 2.0

F32 = mybir.dt.float32
F32R = mybir.dt.float32r
AT = mybir.ActivationFunctionType
OP = mybir.AluOpType

KC = HID // 128              # k-chunks for a 512-wide input
MC = HID // 128              # m-chunks for a 512-wide output


def build_nc(rows=ROWS, rt=RT):
    nrt = rows // rt
    nc = bacc.Bacc("TRN2", target_bir_lowering=False, debug=False,
                   num_devices=N_CORES)

    z = nc.dram_tensor("z", [rows, DIM], F32, kind="ExternalInput")
    w = {}
    for p in ("s", "t"):
        w[p + "W1T"] = nc.dram_tensor(p + "W1T", [128, HID], F32R, kind="ExternalInput")
        w[p + "W2T"] = nc.dram_tensor(p + "W2T", [HID, HID], F32R, kind="ExternalInput")
        w[p + "W3T"] = nc.dram_tensor(p + "W3T", [HID, HID], F32R, kind="ExternalInput")
        w[p + "W4T"] = nc.dram_tensor(p + "W4T", [HID, D_OUT], F32R, kind="ExternalInput")
        w[p + "b1"] = nc.dram_tensor(p + "b1", [HID], F32, kind="ExternalInput")
        w[p + "b2"] = nc.dram_tensor(p + "b2", [HID], F32, kind="ExternalInput")
        w[p + "b3"] = nc.dram_tensor(p + "b3", [HID], F32, kind="ExternalInput")
        w[p + "b4"] = nc.dram_tensor(p + "b4", [D_OUT], F32, kind="ExternalInput")
    z_out = nc.dram_tensor("z_out", [rows, DIM], F32, kind="ExternalOutput")
    log_det = nc.dram_tensor("log_det", [rows], F32, kind="ExternalOutput")

    with tile.TileContext(nc) as tc, ExitStack() as ctx:
        consts = ctx.enter_context(tc.tile_pool(name="consts", bufs=1))
        io = ctx.enter_context(tc.tile_pool(name="io", bufs=4))
        acts = ctx.enter_context(tc.tile_pool(name="acts", bufs=2))
        hpool = ctx.enter_context(tc.tile_pool(name="hpool", bufs=12))
        psum = ctx.enter_context(tc.tile_pool(name="psum", bufs=2, space="PSUM"))

        # --- constants ---
        ident = consts.tile([128, 128], F32, name="ident")
        make_identity(nc, ident)
        ones64_f = consts.tile([D_IN, 1], F32, name="ones64_f")
        nc.vector.memset(ones64_f, 1.0)
        ones64 = consts.tile([D_IN, 1], F32R, name="ones64")
        nc.vector.tensor_copy(ones64, ones64_f)

        cpr0 = rt // 128
        zrow_pre = []
        for pit in range(2):
            zr = io.tile([128, cpr0, DIM], F32, name=f"zrow_{pit}", tag="zrow")
            nc.sync.dma_start(
                out=zr,
                in_=z[pit * rt : (pit + 1) * rt, :].rearrange("(c p) d -> p c d", p=128),
            )
            zrow_pre.append(zr)

        W = {}
        for p in ("s", "t"):
            w1t = consts.tile([128, HID], F32R, name=f"{p}w1t")
            nc.sync.dma_start(out=w1t, in_=w[p + "W1T"][:, :])
            w2t = []
            w3t = []
            w4t = []
            for k in range(KC):
                t2 = consts.tile([128, HID], F32R, name=f"{p}w2t{k}")
                nc.sync.dma_start(out=t2, in_=w[p + "W2T"][128 * k : 128 * (k + 1), :])
                w2t.append(t2)
                t3 = consts.tile([128, HID], F32R, name=f"{p}w3t{k}")
                nc.sync.dma_start(out=t3, in_=w[p + "W3T"][128 * k : 128 * (k + 1), :])
                w3t.append(t3)
                t4 = consts.tile([128, D_OUT], F32R, name=f"{p}w4t{k}")
                nc.sync.dma_start(out=t4, in_=w[p + "W4T"][128 * k : 128 * (k + 1), :])
                w4t.append(t4)
            b1 = consts.tile([128, MC], F32, name=f"{p}b1t")
            nc.sync.dma_start(out=b1, in_=w[p + "b1"][:].rearrange("(m p) -> p m", p=128))
            b2 = consts.tile([128, MC], F32, name=f"{p}b2t")
            nc.sync.dma_start(out=b2, in_=w[p + "b2"][:].rearrange("(m p) -> p m", p=128))
            b3 = consts.tile([128, MC], F32, name=f"{p}b3t")
            nc.sync.dma_start(out=b3, in_=w[p + "b3"][:].rearrange("(m p) -> p m", p=128))
            b4 = consts.tile([D_OUT, 1], F32, name=f"{p}b4t")
            nc.sync.dma_start(out=b4, in_=w[p + "b4"][:].rearrange("(d o) -> d o", o=1))
            W[p] = dict(w1t=w1t, w2t=w2t, w3t=w3t, w4t=w4t, b1=b1, b2=b2, b3=b3, b4=b4)

        cpr = rt // 128  # 128-row chunks per row tile

        half = rt // 2

        def relu_split(h, ps, b_ap):
            """PSUM->SBUF bias+relu split across ScalarE/VectorE for latency."""
            nc.scalar.activation(h[:, 0:half], ps[:, 0:half], AT.Relu, bias=b_ap)
            nc.vector.tensor_scalar(h[:, half:rt], ps[:, half:rt], b_ap, 0.0,
                                    OP.add, OP.max)

        def hidden_layer(wk, b, h_in, relu_on_act, it, p, li):
            """h_out[m] = relu(sum_k wk[k][:,m].T @ h_in[k] + b[:,m]).

            k-outer inside m-pairs: only 2 PSUM banks live per pair, so the
            next group never waits on relu drains."""
            nK = len(h_in)
            h_out = [None] * MC
            for mp in range(0, MC, 2):
                pair = (mp, mp + 1)
                ps = [
                    psum.tile([128, rt], F32, name=f"ps_{p}{li}_{it}_{m}",
                              tag="hps", bufs=5)
                    for m in pair
                ]
                for k in range(nK):
                    for j, m in enumerate(pair):
                        nc.tensor.matmul(ps[j], wk[k][:, 128 * m : 128 * (m + 1)],
                                         h_in[k], start=(k == 0), stop=(k == nK - 1))
                for j, m in enumerate(pair):
                    h = hpool.tile([128, rt], F32R, name=f"h_{p}{li}_{it}_{m}", tag="h")
                    relu_split(h, ps[j], b[:, m : m + 1])
                    h_out[m] = h
            return h_out

        def emit_dma_in(it):
            if it < 2:
                return zrow_pre[it]
            r0 = it * rt
            zrow = io.tile([128, cpr, DIM], F32, name=f"zrow_{it}", tag="zrow")
            nc.sync.dma_start(
                out=zrow, in_=z[r0 : r0 + rt, :].rearrange("(c p) d -> p c d", p=128)
            )
            return zrow

        def emit_A(it, zrow):
            """Input transposes + feature-major copies (PE + DVE, off critical path)."""
            tpz = psum.tile([128, rt], F32, name=f"tpz_{it}", tag="small", bufs=2)
            for c in range(cpr):
                nc.tensor.transpose(tpz[:, 128 * c : 128 * (c + 1)], zrow[:, c, :], ident)
            zT = acts.tile([128, rt], F32R, name=f"zT_{it}", tag="zT", bufs=3)
            nc.vector.tensor_copy(zT, tpz)
            tpz2 = psum.tile([D_OUT, rt], F32, name=f"tpz2_{it}", tag="small", bufs=2)
            for c in range(cpr):
                nc.tensor.transpose(tpz2[:, 128 * c : 128 * (c + 1)],
                                    zrow[:, c, D_IN:DIM], ident)
            z2T = acts.tile([D_OUT, rt], F32, name=f"z2T_{it}", tag="z2T", bufs=3)
            nc.vector.tensor_copy(z2T, tpz2)
            return dict(zrow=zrow, zT=zT, z2T=z2T)

        def emit_L1(it, zT):
            h1 = {}
            for p in ("s", "t"):
                ww = W[p]
                hs = [None] * MC
                for mp in range(0, MC, 2):
                    pair = (mp, mp + 1)
                    ps1 = [
                        psum.tile([128, rt], F32, name=f"ps1_{p}_{it}_{m}",
                                  tag="hps", bufs=5)
                        for m in pair
                    ]
                    for j, m in enumerate(pair):
                        nc.tensor.matmul(ps1[j], ww["w1t"][:, 128 * m : 128 * (m + 1)], zT)
                    for j, m in enumerate(pair):
                        h = hpool.tile([128, rt], F32R, name=f"h1_{p}_{it}_{m}", tag="h")
                        relu_split(h, ps1[j], ww["b1"][:, m : m + 1])
                        hs[m] = h
                h1[p] = hs
            return h1

        def emit_L4(it, p, h3):
            ps4 = psum.tile([D_OUT, rt], F32, name=f"ps4_{p}_{it}", tag="small", bufs=2)
            for k in range(KC):
                nc.tensor.matmul(ps4, W[p]["w4t"][k], h3[k],
                                 start=(k == 0), stop=(k == KC - 1))
            return ps4

        def emit_heads(it, a, outs):
            """s/t heads + elementwise tail — ScalarE/VectorE only, no PE."""
            s_pre = acts.tile([D_OUT, rt], F32, name=f"s_pre_{it}", tag="s_pre")
            nc.scalar.activation(s_pre, outs["s"], AT.Identity, bias=W["s"]["b4"][:, 0:1])
            log_s = acts.tile([D_OUT, rt], F32R, name=f"log_s_{it}", tag="log_s")
            nc.vector.tensor_scalar(log_s, s_pre, CLAMP, -CLAMP, OP.min, OP.max)
            exp_s = acts.tile([D_OUT, rt], F32, name=f"exp_s_{it}", tag="exp_s")
            nc.scalar.activation(exp_s, log_s, AT.Exp)
            tvals = acts.tile([D_OUT, rt], F32, name=f"tvals_{it}", tag="tvals")
            nc.scalar.activation(tvals, outs["t"], AT.Identity, bias=W["t"]["b4"][:, 0:1])
            zmul = acts.tile([D_OUT, rt], F32, name=f"zmul_{it}", tag="zmul")
            nc.vector.tensor_mul(zmul, a["z2T"], exp_s)
            z2o = acts.tile([D_OUT, rt], F32, name=f"z2o_{it}", tag="z2o")
            nc.vector.tensor_add(z2o, zmul, tvals)
            return dict(log_s=log_s, z2o=z2o)

        def emit_C(it, a, hd):
            """log_det reduce + output transposes + stores (deps long since ready)."""
            r0 = it * rt
            ldp = psum.tile([1, rt], F32, name=f"ldp_{it}", tag="mis", bufs=1)
            nc.tensor.matmul(ldp, ones64, hd["log_s"])  # f32r column sum
            ld_sb = acts.tile([1, rt], F32, name=f"ld_sb_{it}", tag="ld")
            nc.vector.tensor_copy(ld_sb, ldp)
            nc.sync.dma_start(out=log_det[r0 : r0 + rt], in_=ld_sb)
            pso = psum.tile([128, cpr * D_OUT], F32, name=f"pso_{it}", tag="mis", bufs=1)
            for c in range(cpr):
                nc.tensor.transpose(
                    pso[:, D_OUT * c : D_OUT * (c + 1)],
                    hd["z2o"][:, 128 * c : 128 * (c + 1)],
                    ident[0:D_OUT, 0:D_OUT],
                )
            zoutt = io.tile([128, cpr, DIM], F32, name=f"zoutt_{it}", tag="zout")
            for c in range(cpr):
                nc.gpsimd.tensor_copy(zoutt[:, c, 0:D_IN], a["zrow"][:, c, 0:D_IN])
                nc.vector.tensor_copy(
                    zoutt[:, c, D_IN:DIM], pso[:, D_OUT * c : D_OUT * (c + 1)]
                )
            nc.sync.dma_start(
                out=z_out[r0 : r0 + rt, :].rearrange("(c p) d -> p c d", p=128),
                in_=zoutt,
            )

        # software pipeline: C(it-1) and A(it+1) are emitted inside tile it's
        # matmul stream so PE always has dense, dependency-ready work.
        zrow0 = emit_dma_in(0)
        a = emit_A(0, zrow0)
        a_next = None
        prevC = None
        for it in range(nrt):
            if it + 1 < nrt:
                zrow_next = emit_dma_in(it + 1)
            h1 = emit_L1(it, a["zT"])
            h2s = hidden_layer(W["s"]["w2t"], W["s"]["b2"], h1["s"], True, it, "s", 2)
            h2t = hidden_layer(W["t"]["w2t"], W["t"]["b2"], h1["t"], True, it, "t", 2)
            if prevC is not None:
                emit_C(*prevC)
                prevC = None
            h3s = hidden_layer(W["s"]["w3t"], W["s"]["b3"], h2s, True, it, "s", 3)
            h3t = hidden_layer(W["t"]["w3t"], W["t"]["b3"], h2t, True, it, "t", 3)
            if it + 1 < nrt:
                a_next = emit_A(it + 1, zrow_next)
            outs = {"s": emit_L4(it, "s", h3s), "t": emit_L4(it, "t", h3t)}
            hd = emit_heads(it, a, outs)
            prevC = (it, a, hd)
            a = a_next
        emit_C(*prevC)

    nc.compile()
    return nc


def _host_inputs(z, s_Win, s_bin, s_Whid, s_bhid, s_Wout, s_bout,
                 t_Win, t_bin, t_Whid, t_bhid, t_Wout, t_bout):
    f = lambda x: np.ascontiguousarray(np.asarray(x, dtype=np.float32))
    shared = {}
    for p, Win, bin_, Whid, bhid, Wout, bout in (
        ("s", s_Win, s_bin, s_Whid, s_bhid, s_Wout, s_bout),
        ("t", t_Win, t_bin, t_Whid, t_bhid, t_Wout, t_bout),
    ):
        w1t = np.zeros((128, HID), np.float32)
        w1t[:D_IN] = np.asarray(Win).T
        shared[p + "W1T"] = w1t
        shared[p + "W2T"] = f(np.asarray(Whid)[0].T)
        shared[p + "W3T"] = f(np.asarray(Whid)[1].T)
        shared[p + "W4T"] = f(np.asarray(Wout).T)
        shared[p + "b1"] = f(bin_)
        shared[p + "b2"] = f(np.asarray(bhid)[0])
        shared[p + "b3"] = f(np.asarray(bhid)[1])
        shared[p + "b4"] = f(bout)
    z = f(z)
    return z, shared


_NC_CACHE = {}


def _get_nc(rows=ROWS, rt=RT):
    key = (rows, rt)
    if key not in _NC_CACHE:
        _NC_CACHE[key] = build_nc(rows, rt)
    return _NC_CACHE[key]


def run_on_device(z, shared, trace=False, **kw):
    nc = _get_nc()
    in_maps = [
        {"z": z[i * ROWS : (i + 1) * ROWS], **shared} for i in range(N_CORES)
    ]
    res = run_bass_kernel_spmd(nc, in_maps, core_ids=list(range(N_CORES)),
                               trace=trace, **kw)
    z_out = np.concatenate([r["z_out"] for r in res.results], axis=0)
    log_det = np.concatenate([r["log_det"] for r in res.results], axis=0)
    return (z_out, log_det), res


def kernel(**inputs):
    z, shared = _host_inputs(**inputs)
    out, _ = run_on_device(z, shared, trace=False)
    return out


# revision 17
# speedup vs baseline: 1.1655x; 1.0061x over previous
"""Affine coupling layer (RealNVP-style) forward on 8 Trainium2 NeuronCores.

Pure data parallel: z is sharded along batch across 8 cores, the small MLP
weights are replicated.  Per core, activations are kept feature-major
([features, rows]) so the four Linear layers chain on the TensorEngine with
no inter-layer transposes; z tiles are transposed in/out with PE-mode
transpose.  Matmuls run in float32r (full PE rate at N>=256); the log_det
reduction runs in plain fp32 for precision.
"""

from contextlib import ExitStack

import numpy as np

import concourse.bass as bass
import concourse.bacc as bacc
import concourse.mybir as mybir
import concourse.tile as tile
from concourse.bass_utils import run_bass_kernel_spmd
from concourse.masks import make_identity

B, DIM, D_IN, HID = 65536, 128, 64, 512
D_OUT = DIM - D_IN
N_CORES = 8
ROWS = B // N_CORES          # rows per core
RT = 512                     # rows per tile (matmul free dim)
CLAMP = 2.0

F32 = mybir.dt.float32
F32R = mybir.dt.float32r
AT = mybir.ActivationFunctionType
OP = mybir.AluOpType

KC = HID // 128              # k-chunks for a 512-wide input
MC = HID // 128              # m-chunks for a 512-wide output


def build_nc(rows=ROWS, rt=RT):
    nrt = rows // rt
    nc = bacc.Bacc("TRN2", target_bir_lowering=False, debug=False,
                   num_devices=N_CORES)

    z = nc.dram_tensor("z", [rows, DIM], F32, kind="ExternalInput")
    w = {}
    for p in ("s", "t"):
        w[p + "W1T"] = nc.dram_tensor(p + "W1T", [128, HID], F32R, kind="ExternalInput")
        w[p + "W2T"] = nc.dram_tensor(p + "W2T", [HID, HID], F32R, kind="ExternalInput")
        w[p + "W3T"] = nc.dram_tensor(p + "W3T", [HID, HID], F32R, kind="ExternalInput")
        w[p + "W4T"] = nc.dram_tensor(p + "W4T", [HID, D_OUT], F32R, kind="ExternalInput")
        w[p + "b1"] = nc.dram_tensor(p + "b1", [HID], F32, kind="ExternalInput")
        w[p + "b2"] = nc.dram_tensor(p + "b2", [HID], F32, kind="ExternalInput")
        w[p + "b3"] = nc.dram_tensor(p + "b3", [HID], F32, kind="ExternalInput")
        w[p + "b4"] = nc.dram_tensor(p + "b4", [D_OUT], F32, kind="ExternalInput")
    z_out = nc.dram_tensor("z_out", [rows, DIM], F32, kind="ExternalOutput")
    log_det = nc.dram_tensor("log_det", [rows], F32, kind="ExternalOutput")

    with tile.TileContext(nc) as tc, ExitStack() as ctx:
        consts = ctx.enter_context(tc.tile_pool(name="consts", bufs=1))
        io = ctx.enter_context(tc.tile_pool(name="io", bufs=4))
        acts = ctx.enter_context(tc.tile_pool(name="acts", bufs=2))
        hpool = ctx.enter_context(tc.tile_pool(name="hpool", bufs=12))
        psum = ctx.enter_context(tc.tile_pool(name="psum", bufs=2, space="PSUM"))

        # --- constants ---
        ident = consts.tile([128, 128], F32, name="ident")
        make_identity(nc, ident)
        ones64_f = consts.tile([D_IN, 1], F32, name="ones64_f")
        nc.vector.memset(ones64_f, 1.0)
        ones64 = consts.tile([D_IN, 1], F32R, name="ones64")
        nc.vector.tensor_copy(ones64, ones64_f)

        # PE warm-up: ~5us of dummy transposes so HAM reaches K=8/8 and the
        # first real matmuls aren't throttled while the first z tiles load.
        warm = psum.tile([128, 128], F32, name="warm", tag="mis", bufs=1)
        for _ in range(20):
            nc.tensor.transpose(warm, ident, ident)

        cpr0 = rt // 128
        zrow_pre = []
        for pit in range(2):
            zr = io.tile([128, cpr0, DIM], F32, name=f"zrow_{pit}", tag="zrow")
            nc.sync.dma_start(
                out=zr,
                in_=z[pit * rt : (pit + 1) * rt, :].rearrange("(c p) d -> p c d", p=128),
            )
            zrow_pre.append(zr)

        W = {"s": {}, "t": {}}
        for p in ("s", "t"):
            w1t = consts.tile([128, HID], F32R, name=f"{p}w1t")
            nc.sync.dma_start(out=w1t, in_=w[p + "W1T"][:, :])
            W[p]["w1t"] = w1t
        for li, nm in ((2, "W2T"), (3, "W3T")):
            for p in ("s", "t"):
                tiles = []
                for k in range(KC):
                    tk = consts.tile([128, HID], F32R, name=f"{p}w{li}t{k}")
                    nc.sync.dma_start(out=tk, in_=w[p + nm][128 * k : 128 * (k + 1), :])
                    tiles.append(tk)
                W[p][f"w{li}t"] = tiles
        for p in ("s", "t"):
            w4t = []
            for k in range(KC):
                t4 = consts.tile([128, D_OUT], F32R, name=f"{p}w4t{k}")
                nc.sync.dma_start(out=t4, in_=w[p + "W4T"][128 * k : 128 * (k + 1), :])
                w4t.append(t4)
            W[p]["w4t"] = w4t
            b1 = consts.tile([128, MC], F32, name=f"{p}b1t")
            nc.sync.dma_start(out=b1, in_=w[p + "b1"][:].rearrange("(m p) -> p m", p=128))
            b2 = consts.tile([128, MC], F32, name=f"{p}b2t")
            nc.sync.dma_start(out=b2, in_=w[p + "b2"][:].rearrange("(m p) -> p m", p=128))
            b3 = consts.tile([128, MC], F32, name=f"{p}b3t")
            nc.sync.dma_start(out=b3, in_=w[p + "b3"][:].rearrange("(m p) -> p m", p=128))
            b4 = consts.tile([D_OUT, 1], F32, name=f"{p}b4t")
            nc.sync.dma_start(out=b4, in_=w[p + "b4"][:].rearrange("(d o) -> d o", o=1))
            W[p].update(b1=b1, b2=b2, b3=b3, b4=b4)

        cpr = rt // 128  # 128-row chunks per row tile

        half = rt // 2

        def relu_split(h, ps, b_ap):
            """PSUM->SBUF bias+relu split across ScalarE/VectorE for latency."""
            nc.scalar.activation(h[:, 0:half], ps[:, 0:half], AT.Relu, bias=b_ap)
            nc.vector.tensor_scalar(h[:, half:rt], ps[:, half:rt], b_ap, 0.0,
                                    OP.add, OP.max)

        def hidden_layer(wk, b, h_in, relu_on_act, it, p, li):
            """h_out[m] = relu(sum_k wk[k][:,m].T @ h_in[k] + b[:,m]).

            k-outer inside m-pairs: only 2 PSUM banks live per pair, so the
            next group never waits on relu drains."""
            nK = len(h_in)
            h_out = [None] * MC
            for mp in range(0, MC, 2):
                pair = (mp, mp + 1)
                ps = [
                    psum.tile([128, rt], F32, name=f"ps_{p}{li}_{it}_{m}",
                              tag="hps", bufs=5)
                    for m in pair
                ]
                for k in range(nK):
                    for j, m in enumerate(pair):
                        nc.tensor.matmul(ps[j], wk[k][:, 128 * m : 128 * (m + 1)],
                                         h_in[k], start=(k == 0), stop=(k == nK - 1))
                for j, m in enumerate(pair):
                    h = hpool.tile([128, rt], F32R, name=f"h_{p}{li}_{it}_{m}", tag="h")
                    relu_split(h, ps[j], b[:, m : m + 1])
                    h_out[m] = h
            return h_out

        def emit_dma_in(it):
            if it < 2:
                return zrow_pre[it]
            r0 = it * rt
            zrow = io.tile([128, cpr, DIM], F32, name=f"zrow_{it}", tag="zrow")
            nc.sync.dma_start(
                out=zrow, in_=z[r0 : r0 + rt, :].rearrange("(c p) d -> p c d", p=128)
            )
            return zrow

        def emit_A(it, zrow):
            """Input transposes + feature-major copies (PE + DVE, off critical path)."""
            tpz = psum.tile([128, rt], F32, name=f"tpz_{it}", tag="small", bufs=2)
            for c in range(cpr):
                nc.tensor.transpose(tpz[:, 128 * c : 128 * (c + 1)], zrow[:, c, :], ident)
            zT = acts.tile([128, rt], F32R, name=f"zT_{it}", tag="zT", bufs=3)
            nc.vector.tensor_copy(zT, tpz)
            tpz2 = psum.tile([D_OUT, rt], F32, name=f"tpz2_{it}", tag="small", bufs=2)
            for c in range(cpr):
                nc.tensor.transpose(tpz2[:, 128 * c : 128 * (c + 1)],
                                    zrow[:, c, D_IN:DIM], ident)
            z2T = acts.tile([D_OUT, rt], F32, name=f"z2T_{it}", tag="z2T", bufs=3)
            nc.vector.tensor_copy(z2T, tpz2)
            return dict(zrow=zrow, zT=zT, z2T=z2T)

        def emit_L1(it, zT):
            h1 = {}
            for p in ("s", "t"):
                ww = W[p]
                hs = [None] * MC
                for mp in range(0, MC, 2):
                    pair = (mp, mp + 1)
                    ps1 = [
                        psum.tile([128, rt], F32, name=f"ps1_{p}_{it}_{m}",
                                  tag="hps", bufs=5)
                        for m in pair
                    ]
                    for j, m in enumerate(pair):
                        nc.tensor.matmul(ps1[j], ww["w1t"][:, 128 * m : 128 * (m + 1)], zT)
                    for j, m in enumerate(pair):
                        h = hpool.tile([128, rt], F32R, name=f"h1_{p}_{it}_{m}", tag="h")
                        relu_split(h, ps1[j], ww["b1"][:, m : m + 1])
                        hs[m] = h
                h1[p] = hs
            return h1

        def emit_L4(it, p, h3):
            ps4 = psum.tile([D_OUT, rt], F32, name=f"ps4_{p}_{it}", tag="small", bufs=2)
            for k in range(KC):
                nc.tensor.matmul(ps4, W[p]["w4t"][k], h3[k],
                                 start=(k == 0), stop=(k == KC - 1))
            return ps4

        def emit_heads(it, a, outs):
            """s/t heads + elementwise tail — ScalarE/VectorE only, no PE."""
            s_pre = acts.tile([D_OUT, rt], F32, name=f"s_pre_{it}", tag="s_pre")
            nc.scalar.activation(s_pre, outs["s"], AT.Identity, bias=W["s"]["b4"][:, 0:1])
            log_s = acts.tile([D_OUT, rt], F32R, name=f"log_s_{it}", tag="log_s")
            nc.vector.tensor_scalar(log_s, s_pre, CLAMP, -CLAMP, OP.min, OP.max)
            exp_s = acts.tile([D_OUT, rt], F32, name=f"exp_s_{it}", tag="exp_s")
            nc.scalar.activation(exp_s, log_s, AT.Exp)
            tvals = acts.tile([D_OUT, rt], F32, name=f"tvals_{it}", tag="tvals")
            nc.scalar.activation(tvals, outs["t"], AT.Identity, bias=W["t"]["b4"][:, 0:1])
            zmul = acts.tile([D_OUT, rt], F32, name=f"zmul_{it}", tag="zmul")
            nc.vector.tensor_mul(zmul, a["z2T"], exp_s)
            z2o = acts.tile([D_OUT, rt], F32, name=f"z2o_{it}", tag="z2o")
            nc.vector.tensor_add(z2o, zmul, tvals)
            return dict(log_s=log_s, z2o=z2o)

        def emit_C(it, a, hd):
            """log_det reduce + output transposes + stores (deps long since ready)."""
            r0 = it * rt
            ldp = psum.tile([1, rt], F32, name=f"ldp_{it}", tag="mis", bufs=1)
            nc.tensor.matmul(ldp, ones64, hd["log_s"])  # f32r column sum
            ld_sb = acts.tile([1, rt], F32, name=f"ld_sb_{it}", tag="ld")
            nc.vector.tensor_copy(ld_sb, ldp)
            nc.sync.dma_start(out=log_det[r0 : r0 + rt], in_=ld_sb)
            pso = psum.tile([128, cpr * D_OUT], F32, name=f"pso_{it}", tag="mis", bufs=1)
            for c in range(cpr):
                nc.tensor.transpose(
                    pso[:, D_OUT * c : D_OUT * (c + 1)],
                    hd["z2o"][:, 128 * c : 128 * (c + 1)],
                    ident[0:D_OUT, 0:D_OUT],
                )
            zoutt = io.tile([128, cpr, DIM], F32, name=f"zoutt_{it}", tag="zout")
            for c in range(cpr):
                nc.gpsimd.tensor_copy(zoutt[:, c, 0:D_IN], a["zrow"][:, c, 0:D_IN])
                nc.vector.tensor_copy(
                    zoutt[:, c, D_IN:DIM], pso[:, D_OUT * c : D_OUT * (c + 1)]
                )
            nc.sync.dma_start(
                out=z_out[r0 : r0 + rt, :].rearrange("(c p) d -> p c d", p=128),
                in_=zoutt,
            )

        # software pipeline: C(it-1) and A(it+1) are emitted inside tile it's
        # matmul stream so PE always has dense, dependency-ready work.
        zrow0 = emit_dma_in(0)
        a = emit_A(0, zrow0)
        a_next = None
        prevC = None
        for it in range(nrt):
            if it + 1 < nrt:
                zrow_next = emit_dma_in(it + 1)
            h1 = emit_L1(it, a["zT"])
            h2s = hidden_layer(W["s"]["w2t"], W["s"]["b2"], h1["s"], True, it, "s", 2)
            h2t = hidden_layer(W["t"]["w2t"], W["t"]["b2"], h1["t"], True, it, "t", 2)
            if prevC is not None:
                emit_C(*prevC)
                prevC = None
            h3s = hidden_layer(W["s"]["w3t"], W["s"]["b3"], h2s, True, it, "s", 3)
            h3t = hidden_layer(W["t"]["w3t"], W["t"]["b3"], h2t, True, it, "t", 3)
            if it + 1 < nrt:
                a_next = emit_A(it + 1, zrow_next)
            outs = {"s": emit_L4(it, "s", h3s), "t": emit_L4(it, "t", h3t)}
            hd = emit_heads(it, a, outs)
            prevC = (it, a, hd)
            a = a_next
        emit_C(*prevC)

    nc.compile()
    return nc


def _host_inputs(z, s_Win, s_bin, s_Whid, s_bhid, s_Wout, s_bout,
                 t_Win, t_bin, t_Whid, t_bhid, t_Wout, t_bout):
    f = lambda x: np.ascontiguousarray(np.asarray(x, dtype=np.float32))
    shared = {}
    for p, Win, bin_, Whid, bhid, Wout, bout in (
        ("s", s_Win, s_bin, s_Whid, s_bhid, s_Wout, s_bout),
        ("t", t_Win, t_bin, t_Whid, t_bhid, t_Wout, t_bout),
    ):
        w1t = np.zeros((128, HID), np.float32)
        w1t[:D_IN] = np.asarray(Win).T
        shared[p + "W1T"] = w1t
        shared[p + "W2T"] = f(np.asarray(Whid)[0].T)
        shared[p + "W3T"] = f(np.asarray(Whid)[1].T)
        shared[p + "W4T"] = f(np.asarray(Wout).T)
        shared[p + "b1"] = f(bin_)
        shared[p + "b2"] = f(np.asarray(bhid)[0])
        shared[p + "b3"] = f(np.asarray(bhid)[1])
        shared[p + "b4"] = f(bout)
    z = f(z)
    return z, shared


_NC_CACHE = {}


def _get_nc(rows=ROWS, rt=RT):
    key = (rows, rt)
    if key not in _NC_CACHE:
        _NC_CACHE[key] = build_nc(rows, rt)
    return _NC_CACHE[key]


def run_on_device(z, shared, trace=False, **kw):
    nc = _get_nc()
    in_maps = [
        {"z": z[i * ROWS : (i + 1) * ROWS], **shared} for i in range(N_CORES)
    ]
    res = run_bass_kernel_spmd(nc, in_maps, core_ids=list(range(N_CORES)),
                               trace=trace, **kw)
    z_out = np.concatenate([r["z_out"] for r in res.results], axis=0)
    log_det = np.concatenate([r["log_det"] for r in res.results], axis=0)
    return (z_out, log_det), res


def kernel(**inputs):
    z, shared = _host_inputs(**inputs)
    out, _ = run_on_device(z, shared, trace=False)
    return out


# revision 19
# speedup vs baseline: 1.2053x; 1.0342x over previous
"""Affine coupling layer (RealNVP-style) forward on 8 Trainium2 NeuronCores.

Pure data parallel: z is sharded along batch across 8 cores, the small MLP
weights are replicated.  Per core, activations are kept feature-major
([features, rows]) so the four Linear layers chain on the TensorEngine with
no inter-layer transposes; z tiles are transposed in/out with PE-mode
transpose.  Matmuls run in float32r (full PE rate at N>=256); the log_det
reduction runs in plain fp32 for precision.
"""

from contextlib import ExitStack

import numpy as np

import concourse.bass as bass
import concourse.bacc as bacc
import concourse.mybir as mybir
import concourse.tile as tile
from concourse.bass_utils import run_bass_kernel_spmd
from concourse.masks import make_identity

B, DIM, D_IN, HID = 65536, 128, 64, 512
D_OUT = DIM - D_IN
N_CORES = 8
ROWS = B // N_CORES          # rows per core
RT = 512                     # rows per tile (matmul free dim)
CLAMP = 2.0

F32 = mybir.dt.float32
F32R = mybir.dt.float32r
AT = mybir.ActivationFunctionType
OP = mybir.AluOpType

KC = HID // 128              # k-chunks for a 512-wide input
MC = HID // 128              # m-chunks for a 512-wide output


def build_nc(rows=ROWS, rt=RT):
    nrt = rows // rt
    nc = bacc.Bacc("TRN2", target_bir_lowering=False, debug=False,
                   num_devices=N_CORES)

    z = nc.dram_tensor("z", [rows, DIM], F32, kind="ExternalInput")
    w = {}
    for p in ("s", "t"):
        w[p + "W1T"] = nc.dram_tensor(p + "W1T", [128, HID], F32R, kind="ExternalInput")
        w[p + "W2T"] = nc.dram_tensor(p + "W2T", [HID, HID], F32R, kind="ExternalInput")
        w[p + "W3T"] = nc.dram_tensor(p + "W3T", [HID, HID], F32R, kind="ExternalInput")
        w[p + "W4T"] = nc.dram_tensor(p + "W4T", [HID, D_OUT], F32R, kind="ExternalInput")
        w[p + "b1"] = nc.dram_tensor(p + "b1", [HID], F32, kind="ExternalInput")
        w[p + "b2"] = nc.dram_tensor(p + "b2", [HID], F32, kind="ExternalInput")
        w[p + "b3"] = nc.dram_tensor(p + "b3", [HID], F32, kind="ExternalInput")
        w[p + "b4"] = nc.dram_tensor(p + "b4", [D_OUT], F32, kind="ExternalInput")
    z_out = nc.dram_tensor("z_out", [rows, DIM], F32, kind="ExternalOutput")
    log_det = nc.dram_tensor("log_det", [rows], F32, kind="ExternalOutput")

    with tile.TileContext(nc) as tc, ExitStack() as ctx:
        consts = ctx.enter_context(tc.tile_pool(name="consts", bufs=1))
        io = ctx.enter_context(tc.tile_pool(name="io", bufs=5))
        acts = ctx.enter_context(tc.tile_pool(name="acts", bufs=2))
        hpool = ctx.enter_context(tc.tile_pool(name="hpool", bufs=12))
        psum = ctx.enter_context(tc.tile_pool(name="psum", bufs=2, space="PSUM"))

        # --- constants ---
        ident = consts.tile([128, 128], F32, name="ident")
        make_identity(nc, ident)
        ones64_f = consts.tile([D_IN, 1], F32, name="ones64_f")
        nc.vector.memset(ones64_f, 1.0)
        ones64 = consts.tile([D_IN, 1], F32R, name="ones64")
        nc.vector.tensor_copy(ones64, ones64_f)

        # PE warm-up: ~5us of dummy transposes so HAM reaches K=8/8 and the
        # first real matmuls aren't throttled while the first z tiles load.
        warm = psum.tile([128, 128], F32, name="warm", tag="mis", bufs=1)
        for _ in range(20):
            nc.tensor.transpose(warm, ident, ident)

        cpr0 = rt // 128
        zrow_pre = []
        for pit in range(min(3, nrt)):
            zr = io.tile([128, cpr0, DIM], F32, name=f"zrow_{pit}", tag="zrow")
            nc.sync.dma_start(
                out=zr,
                in_=z[pit * rt : (pit + 1) * rt, :].rearrange("(c p) d -> p c d", p=128),
            )
            zrow_pre.append(zr)

        W = {"s": {}, "t": {}}
        for p in ("s", "t"):
            w1t = consts.tile([128, HID], F32R, name=f"{p}w1t")
            nc.sync.dma_start(out=w1t, in_=w[p + "W1T"][:, :])
            W[p]["w1t"] = w1t
        for li, nm in ((2, "W2T"), (3, "W3T")):
            for p in ("s", "t"):
                tiles = []
                for k in range(KC):
                    tk = consts.tile([128, HID], F32R, name=f"{p}w{li}t{k}")
                    nc.sync.dma_start(out=tk, in_=w[p + nm][128 * k : 128 * (k + 1), :])
                    tiles.append(tk)
                W[p][f"w{li}t"] = tiles
        for p in ("s", "t"):
            w4t = []
            for k in range(KC):
                t4 = consts.tile([128, D_OUT], F32R, name=f"{p}w4t{k}")
                nc.sync.dma_start(out=t4, in_=w[p + "W4T"][128 * k : 128 * (k + 1), :])
                w4t.append(t4)
            W[p]["w4t"] = w4t
            b1 = consts.tile([128, MC], F32, name=f"{p}b1t")
            nc.sync.dma_start(out=b1, in_=w[p + "b1"][:].rearrange("(m p) -> p m", p=128))
            b2 = consts.tile([128, MC], F32, name=f"{p}b2t")
            nc.sync.dma_start(out=b2, in_=w[p + "b2"][:].rearrange("(m p) -> p m", p=128))
            b3 = consts.tile([128, MC], F32, name=f"{p}b3t")
            nc.sync.dma_start(out=b3, in_=w[p + "b3"][:].rearrange("(m p) -> p m", p=128))
            b4 = consts.tile([D_OUT, 1], F32, name=f"{p}b4t")
            nc.sync.dma_start(out=b4, in_=w[p + "b4"][:].rearrange("(d o) -> d o", o=1))
            W[p].update(b1=b1, b2=b2, b3=b3, b4=b4)

        cpr = rt // 128  # 128-row chunks per row tile

        half = rt // 2

        def relu_split(h, ps, b_ap):
            """PSUM->SBUF bias+relu split across ScalarE/VectorE for latency."""
            nc.scalar.activation(h[:, 0:half], ps[:, 0:half], AT.Relu, bias=b_ap)
            nc.vector.tensor_scalar(h[:, half:rt], ps[:, half:rt], b_ap, 0.0,
                                    OP.add, OP.max)

        def hidden_layer(wk, b, h_in, relu_on_act, it, p, li):
            """h_out[m] = relu(sum_k wk[k][:,m].T @ h_in[k] + b[:,m]).

            k-outer inside m-pairs: only 2 PSUM banks live per pair, so the
            next group never waits on relu drains."""
            nK = len(h_in)
            h_out = [None] * MC
            for mp in range(0, MC, 2):
                pair = (mp, mp + 1)
                ps = [
                    psum.tile([128, rt], F32, name=f"ps_{p}{li}_{it}_{m}",
                              tag="hps", bufs=5)
                    for m in pair
                ]
                for k in range(nK):
                    for j, m in enumerate(pair):
                        nc.tensor.matmul(ps[j], wk[k][:, 128 * m : 128 * (m + 1)],
                                         h_in[k], start=(k == 0), stop=(k == nK - 1))
                for j, m in enumerate(pair):
                    h = hpool.tile([128, rt], F32R, name=f"h_{p}{li}_{it}_{m}", tag="h")
                    relu_split(h, ps[j], b[:, m : m + 1])
                    h_out[m] = h
            return h_out

        def emit_dma_in(it):
            if it < 3:
                return zrow_pre[it]
            r0 = it * rt
            zrow = io.tile([128, cpr, DIM], F32, name=f"zrow_{it}", tag="zrow")
            nc.sync.dma_start(
                out=zrow, in_=z[r0 : r0 + rt, :].rearrange("(c p) d -> p c d", p=128)
            )
            return zrow

        def emit_A(it, zrow):
            """Input transposes + feature-major copies (PE + DVE, off critical path)."""
            tpz = psum.tile([128, rt], F32, name=f"tpz_{it}", tag="small", bufs=2)
            for c in range(cpr):
                nc.tensor.transpose(tpz[:, 128 * c : 128 * (c + 1)], zrow[:, c, :], ident)
            zT = acts.tile([128, rt], F32R, name=f"zT_{it}", tag="zT", bufs=3)
            nc.vector.tensor_copy(zT, tpz)
            tpz2 = psum.tile([D_OUT, rt], F32, name=f"tpz2_{it}", tag="small", bufs=2)
            for c in range(cpr):
                nc.tensor.transpose(tpz2[:, 128 * c : 128 * (c + 1)],
                                    zrow[:, c, D_IN:DIM], ident)
            z2T = acts.tile([D_OUT, rt], F32, name=f"z2T_{it}", tag="z2T", bufs=3)
            nc.vector.tensor_copy(z2T, tpz2)
            return dict(zrow=zrow, zT=zT, z2T=z2T)

        def emit_L1(it, zT):
            h1 = {}
            for p in ("s", "t"):
                ww = W[p]
                hs = [None] * MC
                for mp in range(0, MC, 2):
                    pair = (mp, mp + 1)
                    ps1 = [
                        psum.tile([128, rt], F32, name=f"ps1_{p}_{it}_{m}",
                                  tag="hps", bufs=5)
                        for m in pair
                    ]
                    for j, m in enumerate(pair):
                        nc.tensor.matmul(ps1[j], ww["w1t"][:, 128 * m : 128 * (m + 1)], zT)
                    for j, m in enumerate(pair):
                        h = hpool.tile([128, rt], F32R, name=f"h1_{p}_{it}_{m}", tag="h1", bufs=20)
                        relu_split(h, ps1[j], ww["b1"][:, m : m + 1])
                        hs[m] = h
                h1[p] = hs
            return h1

        def emit_L4(it, p, h3):
            ps4 = psum.tile([D_OUT, rt], F32, name=f"ps4_{p}_{it}", tag="small", bufs=2)
            for k in range(KC):
                nc.tensor.matmul(ps4, W[p]["w4t"][k], h3[k],
                                 start=(k == 0), stop=(k == KC - 1))
            return ps4

        def emit_heads(it, a, outs):
            """s/t heads + elementwise tail — ScalarE/VectorE only, no PE."""
            s_pre = acts.tile([D_OUT, rt], F32, name=f"s_pre_{it}", tag="s_pre")
            nc.scalar.activation(s_pre, outs["s"], AT.Identity, bias=W["s"]["b4"][:, 0:1])
            log_s = acts.tile([D_OUT, rt], F32R, name=f"log_s_{it}", tag="log_s")
            nc.vector.tensor_scalar(log_s, s_pre, CLAMP, -CLAMP, OP.min, OP.max)
            exp_s = acts.tile([D_OUT, rt], F32, name=f"exp_s_{it}", tag="exp_s")
            nc.scalar.activation(exp_s, log_s, AT.Exp)
            tvals = acts.tile([D_OUT, rt], F32, name=f"tvals_{it}", tag="tvals")
            nc.scalar.activation(tvals, outs["t"], AT.Identity, bias=W["t"]["b4"][:, 0:1])
            zmul = acts.tile([D_OUT, rt], F32, name=f"zmul_{it}", tag="zmul")
            nc.vector.tensor_mul(zmul, a["z2T"], exp_s)
            z2o = acts.tile([D_OUT, rt], F32, name=f"z2o_{it}", tag="z2o")
            nc.vector.tensor_add(z2o, zmul, tvals)
            return dict(log_s=log_s, z2o=z2o)

        def emit_C(it, a, hd):
            """log_det reduce + output transposes + stores (deps long since ready)."""
            r0 = it * rt
            ldp = psum.tile([1, rt], F32, name=f"ldp_{it}", tag="mis", bufs=1)
            nc.tensor.matmul(ldp, ones64, hd["log_s"])  # f32r column sum
            ld_sb = acts.tile([1, rt], F32, name=f"ld_sb_{it}", tag="ld")
            nc.vector.tensor_copy(ld_sb, ldp)
            nc.sync.dma_start(out=log_det[r0 : r0 + rt], in_=ld_sb)
            pso = psum.tile([128, cpr * D_OUT], F32, name=f"pso_{it}", tag="mis", bufs=1)
            for c in range(cpr):
                nc.tensor.transpose(
                    pso[:, D_OUT * c : D_OUT * (c + 1)],
                    hd["z2o"][:, 128 * c : 128 * (c + 1)],
                    ident[0:D_OUT, 0:D_OUT],
                )
            zoutt = io.tile([128, cpr, DIM], F32, name=f"zoutt_{it}", tag="zout")
            for c in range(cpr):
                nc.gpsimd.tensor_copy(zoutt[:, c, 0:D_IN], a["zrow"][:, c, 0:D_IN])
                nc.vector.tensor_copy(
                    zoutt[:, c, D_IN:DIM], pso[:, D_OUT * c : D_OUT * (c + 1)]
                )
            nc.sync.dma_start(
                out=z_out[r0 : r0 + rt, :].rearrange("(c p) d -> p c d", p=128),
                in_=zoutt,
            )

        def emit_AL(it):
            """Weight-light stage: input transposes + L1 (only needs w1t)."""
            a = emit_A(it, emit_dma_in(it))
            h1 = emit_L1(it, a["zT"])
            return a, h1

        # software pipeline, 2 tiles deep: transposes+L1 of tiles it+1/it+2 and
        # the output stage of tile it-1 are emitted inside tile it's matmul
        # stream, so PE always has dense dependency-ready work — including
        # through the initial weight-DMA window and every tile boundary.
        pipe = {i: emit_AL(i) for i in range(min(2, nrt))}
        prevC = None
        for it in range(nrt):
            a, h1 = pipe.pop(it)
            h2s = hidden_layer(W["s"]["w2t"], W["s"]["b2"], h1["s"], True, it, "s", 2)
            h2t = hidden_layer(W["t"]["w2t"], W["t"]["b2"], h1["t"], True, it, "t", 2)
            if prevC is not None:
                emit_C(*prevC)
                prevC = None
            h3s = hidden_layer(W["s"]["w3t"], W["s"]["b3"], h2s, True, it, "s", 3)
            h3t = hidden_layer(W["t"]["w3t"], W["t"]["b3"], h2t, True, it, "t", 3)
            if it + 2 < nrt:
                pipe[it + 2] = emit_AL(it + 2)
            outs = {"s": emit_L4(it, "s", h3s), "t": emit_L4(it, "t", h3t)}
            hd = emit_heads(it, a, outs)
            prevC = (it, a, hd)
        emit_C(*prevC)

    nc.compile()
    return nc


def _host_inputs(z, s_Win, s_bin, s_Whid, s_bhid, s_Wout, s_bout,
                 t_Win, t_bin, t_Whid, t_bhid, t_Wout, t_bout):
    f = lambda x: np.ascontiguousarray(np.asarray(x, dtype=np.float32))
    shared = {}
    for p, Win, bin_, Whid, bhid, Wout, bout in (
        ("s", s_Win, s_bin, s_Whid, s_bhid, s_Wout, s_bout),
        ("t", t_Win, t_bin, t_Whid, t_bhid, t_Wout, t_bout),
    ):
        w1t = np.zeros((128, HID), np.float32)
        w1t[:D_IN] = np.asarray(Win).T
        shared[p + "W1T"] = w1t
        shared[p + "W2T"] = f(np.asarray(Whid)[0].T)
        shared[p + "W3T"] = f(np.asarray(Whid)[1].T)
        shared[p + "W4T"] = f(np.asarray(Wout).T)
        shared[p + "b1"] = f(bin_)
        shared[p + "b2"] = f(np.asarray(bhid)[0])
        shared[p + "b3"] = f(np.asarray(bhid)[1])
        shared[p + "b4"] = f(bout)
    z = f(z)
    return z, shared


_NC_CACHE = {}


def _get_nc(rows=ROWS, rt=RT):
    key = (rows, rt)
    if key not in _NC_CACHE:
        _NC_CACHE[key] = build_nc(rows, rt)
    return _NC_CACHE[key]


def run_on_device(z, shared, trace=False, **kw):
    nc = _get_nc()
    in_maps = [
        {"z": z[i * ROWS : (i + 1) * ROWS], **shared} for i in range(N_CORES)
    ]
    res = run_bass_kernel_spmd(nc, in_maps, core_ids=list(range(N_CORES)),
                               trace=trace, **kw)
    z_out = np.concatenate([r["z_out"] for r in res.results], axis=0)
    log_det = np.concatenate([r["log_det"] for r in res.results], axis=0)
    return (z_out, log_det), res


def kernel(**inputs):
    z, shared = _host_inputs(**inputs)
    out, _ = run_on_device(z, shared, trace=False)
    return out
